# revision 1
# baseline (speedup 1.0000x reference)
"""BiLSTM decoder kernel for Trainium2 (Bass/Tile), data-parallel over batch
across 8 NeuronCores.

Contract: kernel(**inputs) takes the FULL unsharded inputs (as produced by
reference.setup_inputs()) and returns the full (256, 6) float32 output.

Strategy (hardcoded for B=256, S=128, V=50000, E=512, H=1024, P=512, O=6):
  - batch sharded 32/core; LSTM/embedding weights replicated (host-side
    transposed/cast to the PE-friendly layouts).
  - per core: embedding rows gathered by indirect DMA, tanh fused into the
    PE-transpose evacuation; input projection x@Wih_f.T+bias precomputed for
    all timesteps into DRAM (bf16); the sequential scan keeps h/c on-chip with
    gates accumulated in PSUM (gate-dim on partitions, batch on the free dim);
    Whh is held as scaled fp8e4m3 (x1024) and descaled during the PSUM+proj
    combine; backward cell needs only one step (b_hs[0]); small head matmuls
    finish on-chip. Output gathered/unsharded host-side.
"""

import numpy as np
from contextlib import ExitStack

import ml_dtypes

import concourse.bass as bass
import concourse.bacc as bacc
import concourse.mybir as mybir
from concourse.bass import ds
from concourse.tile import TileContext
from concourse.masks import make_identity

F32 = mybir.dt.float32
BF16 = mybir.dt.bfloat16
FP8 = mybir.dt.float8e4
I32 = mybir.dt.int32
AF = mybir.ActivationFunctionType
ALU = mybir.AluOpType

# problem shape (hardcoded per contract)
V, E, H, P2, O = 50000, 512, 1024, 512, 6
B, S = 256, 128
N_CORES = 8
Bc = B // N_CORES          # 32 batch rows per core
G4 = 4 * H                 # 4096 gate rows
KE, KH = E // 128, H // 128
M4 = G4 // 128             # 32 gate-row chunks
NSEQ = S * Bc              # 4096 (t-major: n = t*Bc + b)
NI = NSEQ // 128           # 32 gather tiles
T_PER = 16                 # timesteps per proj psum tile
NCH = T_PER * Bc           # 512
J = NSEQ // NCH            # 8
KH2 = 2 * H // 128         # 16
MP = P2 // 128             # 4
WHH_SCALE = 1024.0
SCAN_UNROLL = 2
GB = 8                     # gather batch (tiles per indirect DMA)

_CACHED = {}


def _build_nc():
    nc = bacc.Bacc("TRN2", target_bir_lowering=False, debug=False,
                   num_devices=N_CORES)

    embed_d = nc.dram_tensor("embed", [V, E], F32, kind="ExternalInput")
    idx_d = nc.dram_tensor("idx", [128, NI], I32, kind="ExternalInput")
    wihT_d = nc.dram_tensor("wihT", [KE, 128, G4], BF16, kind="ExternalInput")
    whhT_d = nc.dram_tensor("whhT", [KH, 128, G4], FP8, kind="ExternalInput")
    wihbT_d = nc.dram_tensor("wihbT", [KE, 128, G4], BF16, kind="ExternalInput")
    biasf_d = nc.dram_tensor("biasf", [128, M4], F32, kind="ExternalInput")
    biasb_d = nc.dram_tensor("biasb", [128, M4], F32, kind="ExternalInput")
    wpT_d = nc.dram_tensor("wpT", [KH2, 128, P2], BF16, kind="ExternalInput")
    bp_d = nc.dram_tensor("bp", [128, MP], F32, kind="ExternalInput")
    wcT_d = nc.dram_tensor("wcT", [KP := MP, 128, O], BF16, kind="ExternalInput")
    bc_d = nc.dram_tensor("bc", [128, 1], F32, kind="ExternalInput")
    y_d = nc.dram_tensor("y", [O, Bc], F32, kind="ExternalOutput")

    proj_d = nc.dram_tensor("proj_scratch", [M4, J, 128, NCH], BF16,
                            kind="Internal")

    es = ExitStack()
    with es:
        whh_sb = es.enter_context(nc.sbuf_tensor([128, KH * G4], FP8))
        wsh_sb = es.enter_context(nc.sbuf_tensor([128, KE * G4], BF16))
        xT_sb = es.enter_context(nc.sbuf_tensor([128, KE * NSEQ], BF16))
        wp_sb = es.enter_context(nc.sbuf_tensor([128, KH2 * P2], BF16))
        wc_sb = es.enter_context(nc.sbuf_tensor([128, KP * O], BF16))
        biasf_sb = es.enter_context(nc.sbuf_tensor([128, M4], F32))
        biasb_sb = es.enter_context(nc.sbuf_tensor([128, M4], F32))
        bp_sb = es.enter_context(nc.sbuf_tensor([128, MP], F32))
        bc_sb = es.enter_context(nc.sbuf_tensor([128, 1], F32))
        idx_sb = es.enter_context(nc.sbuf_tensor([128, NI], I32))
        ident = es.enter_context(nc.sbuf_tensor([128, 128], F32))
        h_bf = es.enter_context(nc.sbuf_tensor([128, KH * Bc], BF16))
        hb_bf = es.enter_context(nc.sbuf_tensor([128, KH * Bc], BF16))
        c_sb = es.enter_context(nc.sbuf_tensor([128, 8 * Bc], F32))
        x0_sb = es.enter_context(nc.sbuf_tensor([128, KE * Bc], BF16))
        gates = [es.enter_context(nc.sbuf_tensor(f"gates{i}", [128, 8 * Bc], F32))
                 for i in range(4)]
        acts = [es.enter_context(nc.sbuf_tensor(f"acts{i}", [128, 8 * Bc], F32))
                for i in range(4)]
        tmp1 = es.enter_context(nc.sbuf_tensor([128, 8 * Bc], F32))
        tmp2 = es.enter_context(nc.sbuf_tensor([128, 8 * Bc], F32))
        p1_sb = es.enter_context(nc.sbuf_tensor([128, MP * Bc], BF16))
        out_sb = es.enter_context(nc.sbuf_tensor([128, Bc], F32))

        with TileContext(nc) as tc:
            make_identity(nc, ident[:])
            nc.sync.dma_start(idx_sb[:], idx_d[:])
            for k in range(KH):
                nc.sync.dma_start(whh_sb[:, k * G4:(k + 1) * G4], whhT_d[k])
            nc.sync.dma_start(biasf_sb[:], biasf_d[:])
            nc.sync.dma_start(biasb_sb[:], biasb_d[:])
            for k in range(KH2):
                nc.sync.dma_start(wp_sb[:, k * P2:(k + 1) * P2], wpT_d[k])
            nc.sync.dma_start(bp_sb[:], bp_d[:])
            for k in range(KP):
                nc.sync.dma_start(wc_sb[:, k * O:(k + 1) * O], wcT_d[k])
            nc.sync.dma_start(bc_sb[:], bc_d[:])

            # phase A: Wih_f.T into the shared weight buffer
            for k in range(KE):
                nc.sync.dma_start(wsh_sb[:, k * G4:(k + 1) * G4], wihT_d[k])

            # phase B: gather + PE transpose + fused tanh -> xT (bf16)
            with tc.tile_pool(name="xg", bufs=3) as xg_pool, \
                 tc.tile_pool(name="trp", bufs=4, space="PSUM") as trp_pool:
                for g in range(NI):
                    xg = xg_pool.tile([128, E], F32, name=f"xg{g}", tag="xg")
                    nc.gpsimd.indirect_dma_start(
                        out=xg[:], out_offset=None, in_=embed_d[:],
                        in_offset=bass.IndirectOffsetOnAxis(
                            ap=idx_sb[:, g:g + 1], axis=0))
                    for e in range(KE):
                        trp = trp_pool.tile([128, 128], F32)
                        nc.tensor.transpose(trp[:], xg[:, e * 128:(e + 1) * 128],
                                            ident[:])
                        nc.scalar.activation(
                            xT_sb[:, e * NSEQ + g * 128: e * NSEQ + (g + 1) * 128],
                            trp[:], AF.Tanh)

            # save x0 (t=0 columns) for the backward cell
            for e in range(KE):
                nc.vector.tensor_copy(x0_sb[:, e * Bc:(e + 1) * Bc],
                                      xT_sb[:, e * NSEQ: e * NSEQ + Bc])

            # phase C: input projection -> proj_d (bf16, bias folded)
            with tc.tile_pool(name="pp", bufs=3, space="PSUM") as pp_pool, \
                 tc.tile_pool(name="stg", bufs=4) as stg_pool:
                for m in range(M4):
                    for j in range(J):
                        pp = pp_pool.tile([128, NCH], F32)
                        for k in range(KE):
                            nc.tensor.matmul(
                                pp[:],
                                wsh_sb[:, k * G4 + m * 128: k * G4 + (m + 1) * 128],
                                xT_sb[:, k * NSEQ + j * NCH: k * NSEQ + (j + 1) * NCH],
                                start=(k == 0), stop=(k == KE - 1))
                        stg = stg_pool.tile([128, NCH], BF16)
                        nc.vector.tensor_scalar_add(stg[:], pp[:],
                                                    biasf_sb[:, m:m + 1])
                        nc.sync.dma_start(proj_d[m, j], stg[:])

            # phase E: Wih_b.T replaces Wih_f.T in the shared buffer
            for k in range(KE):
                nc.sync.dma_start(wsh_sb[:, k * G4:(k + 1) * G4], wihbT_d[k])

            # phase F: forward scan
            sp_cm = tc.tile_pool(name="scanps", bufs=1, space="PSUM")
            sp_pool = sp_cm.__enter__()
            psum_g = [sp_pool.tile([128, 8 * Bc], F32, tag=f"ga{i}",
                                   name=f"psga{i}") for i in range(4)]

            def scan_step(t, proj_pool):
                j, tl = t // T_PER, t % T_PER
                projsb = proj_pool.tile([128, M4 * Bc], BF16)
                nc.sync.dma_start(
                    projsb[:].rearrange("p (m b) -> p m b", m=M4),
                    proj_d[:, ds(j, 1), :, ds(tl * Bc, Bc)].rearrange(
                        "m one p b -> p (one m) b"))
                for m in range(M4):
                    G, cc = divmod(m, 8)
                    for k in range(KH):
                        nc.tensor.matmul(
                            psum_g[G][:, cc * Bc:(cc + 1) * Bc],
                            whh_sb[:, k * G4 + m * 128: k * G4 + (m + 1) * 128],
                            h_bf[:, k * Bc:(k + 1) * Bc],
                            start=(k == 0), stop=(k == KH - 1))
                for G in range(4):
                    nc.vector.scalar_tensor_tensor(
                        gates[G][:], psum_g[G][:], 1.0 / WHH_SCALE,
                        projsb[:, G * 8 * Bc:(G + 1) * 8 * Bc], ALU.mult, ALU.add)
                nc.scalar.activation(acts[0][:], gates[0][:], AF.Sigmoid)
                nc.scalar.activation(acts[1][:], gates[1][:], AF.Sigmoid)
                nc.scalar.activation(acts[2][:], gates[2][:], AF.Tanh)
                nc.scalar.activation(acts[3][:], gates[3][:], AF.Sigmoid)
                nc.vector.tensor_mul(tmp1[:], acts[1][:], c_sb[:])
                nc.vector.tensor_mul(tmp2[:], acts[0][:], acts[2][:])
                nc.vector.tensor_add(c_sb[:], tmp1[:], tmp2[:])
                nc.scalar.activation(tmp1[:], c_sb[:], AF.Tanh)
                nc.vector.tensor_mul(h_bf[:], acts[3][:], tmp1[:])

            nc.gpsimd.memset(h_bf[:], 0.0)
            nc.gpsimd.memset(c_sb[:], 0.0)
            with tc.tile_pool(name="projsb", bufs=3) as proj_pool:
                with tc.For_i(0, S, SCAN_UNROLL) as t:
                    for u in range(SCAN_UNROLL):
                        scan_step(t + u, proj_pool)

            # phase G: backward cell (single step, zero state)
            for m in range(M4):
                G, cc = divmod(m, 8)
                for k in range(KE):
                    nc.tensor.matmul(
                        psum_g[G][:, cc * Bc:(cc + 1) * Bc],
                        wsh_sb[:, k * G4 + m * 128: k * G4 + (m + 1) * 128],
                        x0_sb[:, k * Bc:(k + 1) * Bc],
                        start=(k == 0), stop=(k == KE - 1))
            for m in range(M4):
                G, cc = divmod(m, 8)
                nc.vector.tensor_scalar_add(
                    gates[G][:, cc * Bc:(cc + 1) * Bc],
                    psum_g[G][:, cc * Bc:(cc + 1) * Bc], biasb_sb[:, m:m + 1])
            nc.scalar.activation(acts[0][:], gates[0][:], AF.Sigmoid)
            nc.scalar.activation(acts[2][:], gates[2][:], AF.Tanh)
            nc.scalar.activation(acts[3][:], gates[3][:], AF.Sigmoid)
            nc.vector.tensor_mul(tmp2[:], acts[0][:], acts[2][:])
            nc.scalar.activation(tmp1[:], tmp2[:], AF.Tanh)
            nc.vector.tensor_mul(hb_bf[:], acts[3][:], tmp1[:])
            sp_cm.__exit__(None, None, None)

            # phase H: head
            with tc.tile_pool(name="ph", bufs=1, space="PSUM") as ph_pool:
                psum_p1 = ph_pool.tile([128, MP * Bc], F32, tag="p1")
                psum_o = ph_pool.tile([128, Bc], F32, tag="o")
                for m in range(MP):
                    for k in range(KH2):
                        rhs = (h_bf[:, k * Bc:(k + 1) * Bc] if k < KH
                               else hb_bf[:, (k - KH) * Bc:(k - KH + 1) * Bc])
                        nc.tensor.matmul(
                            psum_p1[:, m * Bc:(m + 1) * Bc],
                            wp_sb[:, k * P2 + m * 128: k * P2 + (m + 1) * 128],
                            rhs, start=(k == 0), stop=(k == KH2 - 1))
                for m in range(MP):
                    nc.vector.tensor_scalar_add(
                        p1_sb[:, m * Bc:(m + 1) * Bc],
                        psum_p1[:, m * Bc:(m + 1) * Bc], bp_sb[:, m:m + 1])
                for k in range(KP):
                    nc.tensor.matmul(
                        psum_o[:O, :], wc_sb[:, k * O:(k + 1) * O],
                        p1_sb[:, k * Bc:(k + 1) * Bc],
                        start=(k == 0), stop=(k == KP - 1))
                nc.scalar.activation(out_sb[:O, :], psum_o[:O, :], AF.Sigmoid,
                                     bias=bc_sb[:O, 0:1])
                nc.sync.dma_start(y_d[:], out_sb[:O, :])

    nc.compile()
    return nc


def _prep_in_maps(inputs):
    tobf16 = lambda a: np.asarray(a, dtype=np.float32).astype(ml_dtypes.bfloat16)
    f32 = lambda a: np.asarray(a, np.float32)

    seq = np.asarray(inputs["seq"])
    wihT = tobf16(inputs["Wih_f"]).T.reshape(KE, 128, G4)
    whhT = (f32(inputs["Whh_f"]).T * WHH_SCALE).reshape(KH, 128, G4) \
        .astype(ml_dtypes.float8_e4m3)
    wihbT = tobf16(inputs["Wih_b"]).T.reshape(KE, 128, G4)
    biasf = (f32(inputs["bih_f"]) + f32(inputs["bhh_f"])).reshape(M4, 128).T.copy()
    biasb = (f32(inputs["bih_b"]) + f32(inputs["bhh_b"])).reshape(M4, 128).T.copy()
    wpT = tobf16(inputs["Wp"]).T.reshape(KH2, 128, P2)
    bp = f32(inputs["bp"]).reshape(MP, 128).T.copy()
    wcT = tobf16(inputs["Wc"]).T.reshape(MP, 128, O)
    bc = np.zeros((128, 1), np.float32)
    bc[:O, 0] = f32(inputs["bc"])
    common = dict(embed=f32(inputs["embed"]), wihT=wihT, whhT=whhT, wihbT=wihbT,
                  biasf=biasf, biasb=biasb, wpT=wpT, bp=bp, wcT=wcT, bc=bc)

    nn = np.arange(NSEQ)
    tt, bb = nn // Bc, nn % Bc
    in_maps = []
    for core in range(N_CORES):
        b0 = core * Bc
        idx = seq[b0 + bb, tt].astype(np.int32).reshape(NI, 128).T.copy()
        m = dict(common)
        m["idx"] = idx
        in_maps.append(m)
    return in_maps


def kernel(**inputs) -> np.ndarray:
    from concourse.bass_utils import run_bass_kernel_spmd
    if "nc" not in _CACHED:
        _CACHED["nc"] = _build_nc()
    nc = _CACHED["nc"]
    in_maps = _prep_in_maps(inputs)
    res = run_bass_kernel_spmd(nc, in_maps, core_ids=list(range(N_CORES)))
    out = np.concatenate([res.results[i]["y"].T for i in range(N_CORES)], axis=0)
    return out.astype(np.float32)



# revision 2
# speedup vs baseline: 1.2171x; 1.2171x over previous
"""BiLSTM decoder kernel for Trainium2 (Bass/Tile), data-parallel over batch
across 8 NeuronCores.

Contract: kernel(**inputs) takes the FULL unsharded inputs (as produced by
reference.setup_inputs()) and returns the full (256, 6) float32 output.

Strategy (hardcoded for B=256, S=128, V=50000, E=512, H=1024, P=512, O=6):
  - batch sharded 32/core; LSTM/embedding weights replicated (host-side
    transposed/cast to the PE-friendly layouts).
  - per core: embedding rows gathered by indirect DMA, tanh fused into the
    PE-transpose evacuation; input projection x@Wih_f.T+bias precomputed for
    all timesteps into DRAM (bf16); the sequential scan keeps h/c on-chip with
    gates accumulated in PSUM (gate-dim on partitions, batch on the free dim);
    Whh is held as scaled fp8e4m3 (x1024) and descaled during the PSUM+proj
    combine; backward cell needs only one step (b_hs[0]); small head matmuls
    finish on-chip. Output gathered/unsharded host-side.
"""

import numpy as np
from contextlib import ExitStack

import ml_dtypes

import concourse.bass as bass
import concourse.bacc as bacc
import concourse.mybir as mybir
from concourse.bass import ds
from concourse.tile import TileContext
from concourse.masks import make_identity

F32 = mybir.dt.float32
BF16 = mybir.dt.bfloat16
FP8 = mybir.dt.float8e4
I32 = mybir.dt.int32
AF = mybir.ActivationFunctionType
ALU = mybir.AluOpType

# problem shape (hardcoded per contract)
V, E, H, P2, O = 50000, 512, 1024, 512, 6
B, S = 256, 128
N_CORES = 8
Bc = B // N_CORES          # 32 batch rows per core
G4 = 4 * H                 # 4096 gate rows
KE, KH = E // 128, H // 128
M4 = G4 // 128             # 32 gate-row chunks
NSEQ = S * Bc              # 4096 (t-major: n = t*Bc + b)
NI = NSEQ // 128           # 32 gather tiles
T_PER = 16                 # timesteps per proj psum tile
NCH = T_PER * Bc           # 512
J = NSEQ // NCH            # 8
KH2 = 2 * H // 128         # 16
MP = P2 // 128             # 4
WHH_SCALE = 1024.0
SCAN_UNROLL = 2
GB = 8                     # gather batch (tiles per indirect DMA)

_CACHED = {}


def _build_nc():
    nc = bacc.Bacc("TRN2", target_bir_lowering=False, debug=False,
                   num_devices=N_CORES)

    embed_d = nc.dram_tensor("embed", [V, E], F32, kind="ExternalInput")
    idx_d = nc.dram_tensor("idx", [128, NI], I32, kind="ExternalInput")
    wihT_d = nc.dram_tensor("wihT", [KE, 128, G4], BF16, kind="ExternalInput")
    whhT_d = nc.dram_tensor("whhT", [KH, 128, G4], FP8, kind="ExternalInput")
    wihbT_d = nc.dram_tensor("wihbT", [KE, 128, G4], BF16, kind="ExternalInput")
    biasf_d = nc.dram_tensor("biasf", [128, M4], F32, kind="ExternalInput")
    biasb_d = nc.dram_tensor("biasb", [128, M4], F32, kind="ExternalInput")
    wpT_d = nc.dram_tensor("wpT", [KH2, 128, P2], BF16, kind="ExternalInput")
    bp_d = nc.dram_tensor("bp", [128, MP], F32, kind="ExternalInput")
    wcT_d = nc.dram_tensor("wcT", [KP := MP, 128, O], BF16, kind="ExternalInput")
    bc_d = nc.dram_tensor("bc", [128, 1], F32, kind="ExternalInput")
    y_d = nc.dram_tensor("y", [O, Bc], F32, kind="ExternalOutput")

    proj_d = nc.dram_tensor("proj_scratch", [M4, J, 128, NCH], BF16,
                            kind="Internal")

    es = ExitStack()
    with es:
        whh_sb = es.enter_context(nc.sbuf_tensor([128, KH * G4], FP8))
        wsh_sb = es.enter_context(nc.sbuf_tensor([128, KE * G4], BF16))
        xT_sb = es.enter_context(nc.sbuf_tensor([128, KE * NSEQ], BF16))
        wp_sb = es.enter_context(nc.sbuf_tensor([128, KH2 * P2], BF16))
        wc_sb = es.enter_context(nc.sbuf_tensor([128, KP * O], BF16))
        biasf_sb = es.enter_context(nc.sbuf_tensor([128, M4], F32))
        biasb_sb = es.enter_context(nc.sbuf_tensor([128, M4], F32))
        bp_sb = es.enter_context(nc.sbuf_tensor([128, MP], F32))
        bc_sb = es.enter_context(nc.sbuf_tensor([128, 1], F32))
        idx_sb = es.enter_context(nc.sbuf_tensor([128, NI], I32))
        ident = es.enter_context(nc.sbuf_tensor([128, 128], F32))
        h_bf = es.enter_context(nc.sbuf_tensor([128, KH * Bc], BF16))
        hb_bf = es.enter_context(nc.sbuf_tensor([128, KH * Bc], BF16))
        c_sb = es.enter_context(nc.sbuf_tensor([128, 8 * Bc], F32))
        x0_sb = es.enter_context(nc.sbuf_tensor([128, KE * Bc], BF16))
        gates = [es.enter_context(nc.sbuf_tensor(f"gates{i}", [128, 8 * Bc], F32))
                 for i in range(4)]
        acts = [es.enter_context(nc.sbuf_tensor(f"acts{i}", [128, 8 * Bc], F32))
                for i in range(4)]
        tmp1 = es.enter_context(nc.sbuf_tensor([128, 8 * Bc], F32))
        tmp2 = es.enter_context(nc.sbuf_tensor([128, 8 * Bc], F32))
        p1_sb = es.enter_context(nc.sbuf_tensor([128, MP * Bc], BF16))
        out_sb = es.enter_context(nc.sbuf_tensor([128, Bc], F32))

        with TileContext(nc) as tc:
            make_identity(nc, ident[:])
            nc.sync.dma_start(idx_sb[:], idx_d[:])
            for k in range(KH):
                nc.sync.dma_start(whh_sb[:, k * G4:(k + 1) * G4], whhT_d[k])
            nc.sync.dma_start(biasf_sb[:], biasf_d[:])
            nc.sync.dma_start(biasb_sb[:], biasb_d[:])
            for k in range(KH2):
                nc.sync.dma_start(wp_sb[:, k * P2:(k + 1) * P2], wpT_d[k])
            nc.sync.dma_start(bp_sb[:], bp_d[:])
            for k in range(KP):
                nc.sync.dma_start(wc_sb[:, k * O:(k + 1) * O], wcT_d[k])
            nc.sync.dma_start(bc_sb[:], bc_d[:])

            # phase A: Wih_f.T into the shared weight buffer
            for k in range(KE):
                nc.sync.dma_start(wsh_sb[:, k * G4:(k + 1) * G4], wihT_d[k])

            # phase B: gather + PE transpose + fused tanh -> xT (bf16)
            with tc.tile_pool(name="xg", bufs=3) as xg_pool, \
                 tc.tile_pool(name="trp", bufs=4, space="PSUM") as trp_pool:
                for g in range(NI):
                    xg = xg_pool.tile([128, E], F32, name=f"xg{g}", tag="xg")
                    nc.gpsimd.indirect_dma_start(
                        out=xg[:], out_offset=None, in_=embed_d[:],
                        in_offset=bass.IndirectOffsetOnAxis(
                            ap=idx_sb[:, g:g + 1], axis=0))
                    for e in range(KE):
                        trp = trp_pool.tile([128, 128], F32)
                        nc.tensor.transpose(trp[:], xg[:, e * 128:(e + 1) * 128],
                                            ident[:])
                        nc.scalar.activation(
                            xT_sb[:, e * NSEQ + g * 128: e * NSEQ + (g + 1) * 128],
                            trp[:], AF.Tanh)

            # save x0 (t=0 columns) for the backward cell
            for e in range(KE):
                nc.vector.tensor_copy(x0_sb[:, e * Bc:(e + 1) * Bc],
                                      xT_sb[:, e * NSEQ: e * NSEQ + Bc])

            # phase C: input projection -> proj_d (bf16, bias folded)
            with tc.tile_pool(name="pp", bufs=3, space="PSUM") as pp_pool, \
                 tc.tile_pool(name="stg", bufs=4) as stg_pool:
                for m in range(M4):
                    for j in range(J):
                        pp = pp_pool.tile([128, NCH], F32)
                        for k in range(KE):
                            nc.tensor.matmul(
                                pp[:],
                                wsh_sb[:, k * G4 + m * 128: k * G4 + (m + 1) * 128],
                                xT_sb[:, k * NSEQ + j * NCH: k * NSEQ + (j + 1) * NCH],
                                start=(k == 0), stop=(k == KE - 1))
                        stg = stg_pool.tile([128, NCH], BF16)
                        nc.vector.tensor_scalar_add(stg[:], pp[:],
                                                    biasf_sb[:, m:m + 1])
                        nc.sync.dma_start(proj_d[m, j], stg[:])

            # phase E: Wih_b.T replaces Wih_f.T in the shared buffer
            for k in range(KE):
                nc.sync.dma_start(wsh_sb[:, k * G4:(k + 1) * G4], wihbT_d[k])

            # phase F: forward scan
            sp_cm = tc.tile_pool(name="scanps", bufs=1, space="PSUM")
            sp_pool = sp_cm.__enter__()
            psum_g = [sp_pool.tile([128, 8 * Bc], F32, tag=f"ga{i}",
                                   name=f"psga{i}") for i in range(4)]

            def scan_step(t, proj_pool):
                # Pipelined step: MMs in 4 blocks (k-half x cc-half); the
                # chain for h-half A runs under block 4; next step's k0-3
                # blocks only need half A, so half B's chain hides under them.
                j, tl = t // T_PER, t % T_PER
                projsb = proj_pool.tile([128, M4 * Bc], BF16)
                nc.sync.dma_start(
                    projsb[:].rearrange("p (m b) -> p m b", m=M4),
                    proj_d[:, ds(j, 1), :, ds(tl * Bc, Bc)].rearrange(
                        "m one p b -> p (one m) b"))

                def mm_block(ks, ccs):
                    for k in ks:
                        for cc in ccs:
                            for G in range(4):
                                m = G * 8 + cc
                                nc.tensor.matmul(
                                    psum_g[G][:, cc * Bc:(cc + 1) * Bc],
                                    whh_sb[:, k * G4 + m * 128:
                                           k * G4 + (m + 1) * 128],
                                    h_bf[:, k * Bc:(k + 1) * Bc],
                                    start=(k == 0), stop=(k == KH - 1))

                def chain(X):
                    sl = slice(X * 4 * Bc, (X + 1) * 4 * Bc)
                    for G in range(4):
                        nc.vector.scalar_tensor_tensor(
                            gates[G][:, sl], psum_g[G][:, sl], 1.0 / WHH_SCALE,
                            projsb[:, (G * 8 + X * 4) * Bc:
                                   (G * 8 + X * 4 + 4) * Bc],
                            ALU.mult, ALU.add)
                    nc.scalar.activation(acts[0][:, sl], gates[0][:, sl],
                                         AF.Sigmoid)
                    nc.scalar.activation(acts[1][:, sl], gates[1][:, sl],
                                         AF.Sigmoid)
                    nc.scalar.activation(acts[2][:, sl], gates[2][:, sl],
                                         AF.Tanh)
                    nc.scalar.activation(acts[3][:, sl], gates[3][:, sl],
                                         AF.Sigmoid)
                    nc.vector.tensor_mul(tmp1[:, sl], acts[1][:, sl],
                                         c_sb[:, sl])
                    nc.vector.tensor_mul(tmp2[:, sl], acts[0][:, sl],
                                         acts[2][:, sl])
                    nc.vector.tensor_add(c_sb[:, sl], tmp1[:, sl], tmp2[:, sl])
                    nc.scalar.activation(tmp1[:, sl], c_sb[:, sl], AF.Tanh)
                    nc.vector.tensor_mul(h_bf[:, sl], acts[3][:, sl],
                                         tmp1[:, sl])

                mm_block(range(0, 4), range(0, 4))
                mm_block(range(0, 4), range(4, 8))
                mm_block(range(4, 8), range(0, 4))
                chain(0)
                mm_block(range(4, 8), range(4, 8))
                chain(1)

            nc.gpsimd.memset(h_bf[:], 0.0)
            nc.gpsimd.memset(c_sb[:], 0.0)
            with tc.tile_pool(name="projsb", bufs=3) as proj_pool:
                with tc.For_i(0, S, SCAN_UNROLL) as t:
                    for u in range(SCAN_UNROLL):
                        scan_step(t + u, proj_pool)

            # phase G: backward cell (single step, zero state)
            for m in range(M4):
                G, cc = divmod(m, 8)
                for k in range(KE):
                    nc.tensor.matmul(
                        psum_g[G][:, cc * Bc:(cc + 1) * Bc],
                        wsh_sb[:, k * G4 + m * 128: k * G4 + (m + 1) * 128],
                        x0_sb[:, k * Bc:(k + 1) * Bc],
                        start=(k == 0), stop=(k == KE - 1))
            for m in range(M4):
                G, cc = divmod(m, 8)
                nc.vector.tensor_scalar_add(
                    gates[G][:, cc * Bc:(cc + 1) * Bc],
                    psum_g[G][:, cc * Bc:(cc + 1) * Bc], biasb_sb[:, m:m + 1])
            nc.scalar.activation(acts[0][:], gates[0][:], AF.Sigmoid)
            nc.scalar.activation(acts[2][:], gates[2][:], AF.Tanh)
            nc.scalar.activation(acts[3][:], gates[3][:], AF.Sigmoid)
            nc.vector.tensor_mul(tmp2[:], acts[0][:], acts[2][:])
            nc.scalar.activation(tmp1[:], tmp2[:], AF.Tanh)
            nc.vector.tensor_mul(hb_bf[:], acts[3][:], tmp1[:])
            sp_cm.__exit__(None, None, None)

            # phase H: head
            with tc.tile_pool(name="ph", bufs=1, space="PSUM") as ph_pool:
                psum_p1 = ph_pool.tile([128, MP * Bc], F32, tag="p1")
                psum_o = ph_pool.tile([128, Bc], F32, tag="o")
                for m in range(MP):
                    for k in range(KH2):
                        rhs = (h_bf[:, k * Bc:(k + 1) * Bc] if k < KH
                               else hb_bf[:, (k - KH) * Bc:(k - KH + 1) * Bc])
                        nc.tensor.matmul(
                            psum_p1[:, m * Bc:(m + 1) * Bc],
                            wp_sb[:, k * P2 + m * 128: k * P2 + (m + 1) * 128],
                            rhs, start=(k == 0), stop=(k == KH2 - 1))
                for m in range(MP):
                    nc.vector.tensor_scalar_add(
                        p1_sb[:, m * Bc:(m + 1) * Bc],
                        psum_p1[:, m * Bc:(m + 1) * Bc], bp_sb[:, m:m + 1])
                for k in range(KP):
                    nc.tensor.matmul(
                        psum_o[:O, :], wc_sb[:, k * O:(k + 1) * O],
                        p1_sb[:, k * Bc:(k + 1) * Bc],
                        start=(k == 0), stop=(k == KP - 1))
                nc.scalar.activation(out_sb[:O, :], psum_o[:O, :], AF.Sigmoid,
                                     bias=bc_sb[:O, 0:1])
                nc.sync.dma_start(y_d[:], out_sb[:O, :])

    nc.compile()
    return nc


def _prep_in_maps(inputs):
    tobf16 = lambda a: np.asarray(a, dtype=np.float32).astype(ml_dtypes.bfloat16)
    f32 = lambda a: np.asarray(a, np.float32)

    seq = np.asarray(inputs["seq"])
    wihT = tobf16(inputs["Wih_f"]).T.reshape(KE, 128, G4)
    whhT = (f32(inputs["Whh_f"]).T * WHH_SCALE).reshape(KH, 128, G4) \
        .astype(ml_dtypes.float8_e4m3)
    wihbT = tobf16(inputs["Wih_b"]).T.reshape(KE, 128, G4)
    biasf = (f32(inputs["bih_f"]) + f32(inputs["bhh_f"])).reshape(M4, 128).T.copy()
    biasb = (f32(inputs["bih_b"]) + f32(inputs["bhh_b"])).reshape(M4, 128).T.copy()
    wpT = tobf16(inputs["Wp"]).T.reshape(KH2, 128, P2)
    bp = f32(inputs["bp"]).reshape(MP, 128).T.copy()
    wcT = tobf16(inputs["Wc"]).T.reshape(MP, 128, O)
    bc = np.zeros((128, 1), np.float32)
    bc[:O, 0] = f32(inputs["bc"])
    common = dict(embed=f32(inputs["embed"]), wihT=wihT, whhT=whhT, wihbT=wihbT,
                  biasf=biasf, biasb=biasb, wpT=wpT, bp=bp, wcT=wcT, bc=bc)

    nn = np.arange(NSEQ)
    tt, bb = nn // Bc, nn % Bc
    in_maps = []
    for core in range(N_CORES):
        b0 = core * Bc
        idx = seq[b0 + bb, tt].astype(np.int32).reshape(NI, 128).T.copy()
        m = dict(common)
        m["idx"] = idx
        in_maps.append(m)
    return in_maps


def kernel(**inputs) -> np.ndarray:
    from concourse.bass_utils import run_bass_kernel_spmd
    if "nc" not in _CACHED:
        _CACHED["nc"] = _build_nc()
    nc = _CACHED["nc"]
    in_maps = _prep_in_maps(inputs)
    res = run_bass_kernel_spmd(nc, in_maps, core_ids=list(range(N_CORES)))
    out = np.concatenate([res.results[i]["y"].T for i in range(N_CORES)], axis=0)
    return out.astype(np.float32)

